# revision 25
# baseline (speedup 1.0000x reference)
"""Causal self-attention (B=2, T=2048, D=2048, 16 heads) on 8 NeuronCores.

Tensor-parallel over heads: core c owns heads {2c, 2c+1}. Each core computes
its heads' Q/K/V projections, causal attention, and a partial output
projection (row-parallel); the host sums the 8 partials.

Quantized-matmul strategy (vs the fp32r baseline):
  - QKV and output projections run as fp8e4 DoubleRow matmuls (2 contraction
    planes per instruction, 0.5 cyc/row) using a 3-term hi/lo split:
    A@B ~= Ah@Bh + Al@Bh + Ah@Bl with the lo*lo term dropped. Each term
    pairs two 128-deep contraction planes per instruction, so the projection
    costs 0.75 cyc/row vs 1.0 for fp32r. Weights are pre-scaled by SW=32
    (power of two) before the hi/lo split so their ~1/sqrt(D) magnitudes sit
    in fp8's normal range; the scale is divided out exactly in the final
    y copy (1/SW^2) and the exp scale (1/SW^2).
  - S = K^T Q stays fp32r (exact; q/k land 32-scaled in f32).
  - P = exp(S) is emitted directly in f16 by the ACT engine; V is stored
    f16 (32-scaled), so PV runs f16 at the same PE cost as fp32r with half
    the SBUF/PSUM traffic. The softmax denominator accumulates on DVE in
    f16 (2x element rate; partials bounded ~3e4 < f16 max) and folds via
    PE ones-matmuls.
  - The attention output o (normalized, 32-scaled, |o|<=~140 < fp8e4 max
    240) is split hi/lo on the fly (Pool copy + DVE sub) for the fp8
    output projection.
  - x arrives as host-converted fp8 hi/lo planes (half the input DMA);
    y partials leave as f16 (half the output DMA); host sums in f32.
"""
import numpy as np
from contextlib import ExitStack

import concourse.bass as bass
import concourse.tile as tile
from concourse import bacc
from concourse import mybir
from concourse.bass_utils import run_bass_kernel_spmd

f32 = mybir.dt.float32
f32r = mybir.dt.float32r
bf16 = mybir.dt.bfloat16
f16 = mybir.dt.float16
fp8 = mybir.dt.float8e4
DR = mybir.MatmulPerfMode.DoubleRow

B, T, D = 2, 2048, 2048
H, HD = 16, 128
N_CORES = 8
NH = H // N_CORES            # heads per core = 2
SCALE = float(HD) ** -0.5    # 1/sqrt(128)
NEG = -1.0e9
SW = 32.0                    # weight pre-scale (power of two)
EXP_SCALE = SCALE / (SW * SW)
Y_SCALE = 1.0 / (SW * SW)

DT = D // 128                # 16 D-tiles (contraction)
CH = 256                     # token chunk for QKV projection
NCH = T // CH                # 8 chunks per batch
TT = T // 128                # 16 token tiles per batch
QB = 512                     # query block for attention
NQB = T // QB                # 4
EB = 256                     # out-proj feature block (DR moving limit)


def _body(ctx, tc, xh, xl, wqh, wql, woh, wol, mask, y):
    nc = tc.nc

    singles = ctx.enter_context(tc.tile_pool(name="singles", bufs=1))
    wqh_sb = singles.tile([128, DT, 3 * NH * HD], fp8)
    wql_sb = singles.tile([128, DT, 3 * NH * HD], fp8)
    wqh_r = wqh.rearrange("(n p) f -> p n f", p=128)
    wql_r = wql.rearrange("(n p) f -> p n f", p=128)
    # wo/mask are loaded later (first needed at attention/out-proj time).
    woh_sb = singles.tile([128, NH, D], fp8)
    wol_sb = singles.tile([128, NH, D], fp8)
    mask_sb = singles.tile([128, 128], f32)
    ones_col = singles.tile([128, 1], f16)
    nc.vector.memset(ones_col, 1.0)
    ones_row_f = singles.tile([1, 128], f32)
    nc.vector.memset(ones_row_f, 1.0)
    ones_row = singles.tile([1, 128], f32r)
    nc.vector.tensor_copy(ones_row, ones_row_f)
    zeros_sb = singles.tile([128, 128], f16)
    nc.vector.memset(zeros_sb, 0.0)

    perbatch = ctx.enter_context(tc.tile_pool(name="perbatch", bufs=1))
    xpool = ctx.enter_context(tc.tile_pool(name="xpool", bufs=3))
    attsb = ctx.enter_context(tc.tile_pool(name="attsb", bufs=4))
    densb = ctx.enter_context(tc.tile_pool(name="densb", bufs=2))
    smallsb = ctx.enter_context(tc.tile_pool(name="smallsb", bufs=2))
    oslsb = ctx.enter_context(tc.tile_pool(name="oslsb", bufs=2))
    ysb_pool = ctx.enter_context(tc.tile_pool(name="ysb", bufs=7))
    # PSUM budget: A{qkv acc, y} 1 bank x2 + B{s2 pairs} 2 banks x2 +
    # O{attention out} 1 bank x1 + C{den, rb} 1 bank x1 = 8 banks.
    ps = ctx.enter_context(tc.tile_pool(name="ps", bufs=2, space="PSUM"))

    for b in range(B):
        qt_sb = perbatch.tile([128, NH, T], f32r, tag="qt")    # Q^T (32-scaled)
        kt_sb = perbatch.tile([128, NH, T], f32r, tag="kt")    # K^T (32-scaled)
        v_sb = perbatch.tile([128, NH, TT, HD], f16, tag="v")  # V (32-scaled)
        oh_sb = perbatch.tile([128, NH, T], fp8, tag="oh")     # out^T hi
        ol_sb = perbatch.tile([128, NH, T], fp8, tag="ol")     # out^T lo

        # QKV projection for one token chunk: six sequential accumulation
        # groups (q_h0, q_h1, k_h0, k_h1, v_tt0, v_tt1), each a 3-term
        # hi/lo fp8 DoubleRow chain over 8 D-tile pairs.
        xh_b = xh[b].rearrange("(n p) t -> p n t", p=128)
        xl_b = xl[b].rearrange("(n p) t -> p n t", p=128)
        groups = [("q", 0), ("q", 1), ("k", 0), ("k", 1)] + \
                 [("v", tt) for tt in range(CH // 128)]

        chunk_tiles = {}

        def load_chunk(ci):
            xh_ch = xpool.tile([128, DT, CH], fp8, tag="xh")
            xl_ch = xpool.tile([128, DT, CH], fp8, tag="xl")
            chunk_tiles[ci] = (xh_ch, xl_ch)
            srch = xh_b[:, :, ci * CH:(ci + 1) * CH]
            srcl = xl_b[:, :, ci * CH:(ci + 1) * CH]
            if b == 0 and ci == 0:
                # Critical first loads: x chunk 0 and the weight sections in
                # exactly the order the six accumulation groups consume them
                # (q0, q1, k0, k1, v), so each group unblocks as early as
                # possible while later chunks stream behind.
                nc.sync.dma_start(out=xh_ch[:, 0:8, :], in_=srch[:, 0:8, :])
                nc.sync.dma_start(out=wqh_sb[:, :, 0:128],
                                  in_=wqh_r[:, :, 0:128])
                nc.sync.dma_start(out=xh_ch[:, 8:, :], in_=srch[:, 8:, :])
                nc.sync.dma_start(out=wql_sb[:, :, 0:128],
                                  in_=wql_r[:, :, 0:128])
                nc.sync.dma_start(out=xl_ch, in_=srcl)
                for f0, f1 in ((128, 256), (256, 384), (384, 512),
                               (512, 640), (640, 768)):
                    nc.sync.dma_start(out=wqh_sb[:, :, f0:f1],
                                      in_=wqh_r[:, :, f0:f1])
                    nc.sync.dma_start(out=wql_sb[:, :, f0:f1],
                                      in_=wql_r[:, :, f0:f1])
            else:
                # Halve the chunk loads so accumulation groups can start on
                # partial data (region-precise deps) during DMA-starved spans.
                for q8_ in range(0, DT, 8):
                    nc.sync.dma_start(out=xh_ch[:, q8_:q8_ + 8, :],
                                      in_=srch[:, q8_:q8_ + 8, :])
                for q8_ in range(0, DT, 8):
                    nc.sync.dma_start(out=xl_ch[:, q8_:q8_ + 8, :],
                                      in_=srcl[:, q8_:q8_ + 8, :])

        def qkv_chunk(ci):
            xh_ch, xl_ch = chunk_tiles.pop(ci)
            for kind, idx in groups:
                acc = ps.tile([128, CH], f32, tag="A")
                n_mm = (DT // 2) * 3
                mi = 0
                for dp in range(0, DT, 2):
                    if kind == "v":
                        tok = slice(idx * 128, (idx + 1) * 128)
                        vcols = slice(2 * NH * HD, 3 * NH * HD)
                        terms = [
                            (xh_ch[:, dp:dp + 2, tok], wqh_sb[:, dp:dp + 2, vcols]),
                            (xl_ch[:, dp:dp + 2, tok], wqh_sb[:, dp:dp + 2, vcols]),
                            (xh_ch[:, dp:dp + 2, tok], wql_sb[:, dp:dp + 2, vcols]),
                        ]
                    else:
                        base = 0 if kind == "q" else NH * HD
                        wcols = slice(base + idx * HD, base + (idx + 1) * HD)
                        terms = [
                            (wqh_sb[:, dp:dp + 2, wcols], xh_ch[:, dp:dp + 2, :]),
                            (wql_sb[:, dp:dp + 2, wcols], xh_ch[:, dp:dp + 2, :]),
                            (wqh_sb[:, dp:dp + 2, wcols], xl_ch[:, dp:dp + 2, :]),
                        ]
                    for lhsT, rhs in terms:
                        nc.tensor.matmul(acc, lhsT=lhsT, rhs=rhs,
                                         start=(mi == 0), stop=(mi == n_mm - 1),
                                         perf_mode=DR)
                        mi += 1
                cols = slice(ci * CH, (ci + 1) * CH)
                if kind == "q":
                    nc.vector.tensor_copy(qt_sb[:, idx, cols], acc)
                elif kind == "k":
                    nc.scalar.copy(kt_sb[:, idx, cols], acc)
                else:
                    nc.vector.tensor_copy(
                        v_sb[:, :, ci * (CH // 128) + idx, :],
                        acc.rearrange("p (h d) -> p h d", h=NH))

        def out_proj(qb, half=None):
            t0 = qb * (QB // 128)
            tis = range(t0, t0 + 4) if half is None else \
                range(t0 + 2 * half, t0 + 2 * half + 2)
            for ti in tis:
                tsl = slice(ti * 128, (ti + 1) * 128)
                for ebw in range(D // 512):
                    y_tile = ysb_pool.tile([128, 512], f16, tag="yt")
                    for sub in range(2):
                        eb = ebw * 2 + sub
                        esl = slice(eb * EB, (eb + 1) * EB)
                        y_ps = ps.tile([128, EB], f32, tag="A")
                        terms = [
                            (oh_sb[:, :, tsl], woh_sb[:, :, esl]),
                            (ol_sb[:, :, tsl], woh_sb[:, :, esl]),
                            (oh_sb[:, :, tsl], wol_sb[:, :, esl]),
                        ]
                        for mi, (lhsT, rhs) in enumerate(terms):
                            nc.tensor.matmul(y_ps, lhsT=lhsT, rhs=rhs,
                                             start=(mi == 0), stop=(mi == 2),
                                             perf_mode=DR)
                        dst = y_tile[:, sub * EB:(sub + 1) * EB]
                        # Split the PSUM-evacuating y copies between DVE and
                        # ACT so neither the den-add stream nor the exp stream
                        # is starved during the attention k-loop.
                        if (ti * 8 + eb) % 2 == 0:
                            nc.vector.tensor_scalar_mul(dst, y_ps, Y_SCALE)
                        else:
                            nc.scalar.mul(dst, y_ps, Y_SCALE)
                    nc.sync.dma_start(
                        out=y[b * T + ti * 128:b * T + (ti + 1) * 128,
                              ebw * 512:(ebw + 1) * 512],
                        in_=y_tile)

        def attention(qb, h):
                den_eng = nc.vector
                nk = (qb + 1) * QB // 128
                o_ps = ps.tile([128, QB], f32, tag="O", bufs=1)
                # Double accumulator: one wide DVE op per k-tile pair; the
                # two halves are folded by the PE ones-matmul reduction.
                den2 = densb.tile([128, 2, QB], f16, tag="den")
                q_slice = qt_sb[:, h, qb * QB:(qb + 1) * QB]
                for p in range(nk // 2):
                    s2 = ps.tile([128, 2, QB], f32, tag="B")
                    pt2 = attsb.tile([128, 2, QB], f16, tag="pt")
                    for j in range(2):
                        kt = 2 * p + j
                        nc.tensor.matmul(
                            s2[:, j, :],
                            lhsT=kt_sb[:, h, kt * 128:(kt + 1) * 128],
                            rhs=q_slice, start=True, stop=True)
                    k_rel0 = (2 * p) * 128 - qb * QB
                    diag = k_rel0 >= 0
                    if diag:
                        # Diagonal pair: triangle mask, then per-subtile exp
                        # restricted to the valid column range. Columns below
                        # the diagonal are never read downstream (PV and den
                        # are restricted the same way), so no memset needed.
                        for j in range(2):
                            kr = k_rel0 + j * 128
                            nc.vector.tensor_add(
                                s2[:, j, kr:kr + 128], s2[:, j, kr:kr + 128],
                                mask_sb)
                            nc.scalar.activation(
                                pt2[:, j, kr:], s2[:, j, kr:],
                                mybir.ActivationFunctionType.Exp,
                                scale=EXP_SCALE)
                    else:
                        nc.scalar.activation(
                            pt2, s2, mybir.ActivationFunctionType.Exp,
                            scale=EXP_SCALE)
                    for j in range(2):
                        kt = 2 * p + j
                        kr = max(k_rel0 + j * 128, 0) if diag else 0
                        nc.tensor.matmul(
                            o_ps[:, kr:], lhsT=v_sb[:, h, kt, :],
                            rhs=pt2[:, j, kr:],
                            start=(kt == 0), stop=(kt == nk - 1))
                    if p == 0:
                        if diag:
                            # qb == 0: j=0 is full width (kr=0); j=1 starts
                            # at column 128 — zero-fill the gap so the PE
                            # fold below reads initialized data.
                            nc.gpsimd.tensor_copy(den2[:, 0, :], pt2[:, 0, :])
                            nc.gpsimd.tensor_copy(den2[:, 1, 128:],
                                                  pt2[:, 1, 128:])
                            nc.vector.tensor_copy(den2[:, 1, 0:128], zeros_sb)
                        else:
                            # 1-input copy runs near line-rate on GpSimd
                            # (P12), keeping the chain head off the busy DVE.
                            nc.gpsimd.tensor_copy(den2, pt2)
                    elif diag:
                        for j in range(2):
                            kr = k_rel0 + j * 128
                            den_eng.tensor_add(den2[:, j, kr:],
                                               den2[:, j, kr:],
                                               pt2[:, j, kr:])
                    else:
                        den_eng.tensor_add(den2, den2, pt2)
                den_ps = ps.tile([1, QB], f32, tag="C", bufs=1)
                for j in range(2):
                    nc.tensor.matmul(den_ps, lhsT=ones_col, rhs=den2[:, j, :],
                                     start=(j == 0), stop=(j == 1))
                recip = smallsb.tile([1, QB], f32r, tag="rcp")
                nc.vector.reciprocal(recip, den_ps)
                rb_ps = ps.tile([128, QB], f32, tag="C", bufs=1)
                nc.tensor.matmul(rb_ps, lhsT=ones_row, rhs=recip,
                                 start=True, stop=True)
                # Normalize (32-scaled, |o| <= ~140), then split hi/lo fp8
                # for the DoubleRow output projection.
                qsl = slice(qb * QB, (qb + 1) * QB)
                osl = oslsb.tile([128, QB], f32r, tag="osl")
                nc.scalar.copy(osl, o_ps)
                nc.vector.tensor_mul(osl, osl, rb_ps)
                nc.gpsimd.tensor_copy(oh_sb[:, h, qsl], osl)
                nc.vector.tensor_sub(ol_sb[:, h, qsl], osl, oh_sb[:, h, qsl])

        # Interleave: attention for query block qb only needs the first
        # 2*qb+2 QKV chunks, so QKV (pure PE) overlaps attention's DVE/ACT
        # load; the output projection lags one block so the denominator
        # chain of block qb overlaps block qb+1's k-loop.
        load_chunk(0)
        load_chunk(1)
        load_chunk(2)
        for c in range(NQB):
            qkv_chunk(2 * c)
            if 2 * c + 3 < 2 * NQB:
                load_chunk(2 * c + 3)
            qkv_chunk(2 * c + 1)
            if 2 * c + 4 < 2 * NQB:
                load_chunk(2 * c + 4)
            if b == 0 and c == 0:
                nc.sync.dma_start(out=mask_sb, in_=mask[:, :])
            if b == 0 and c == 1:
                nc.sync.dma_start(
                    out=woh_sb, in_=woh.rearrange("(n p) e -> p n e", p=128))
                nc.sync.dma_start(
                    out=wol_sb, in_=wol.rearrange("(n p) e -> p n e", p=128))
            attention(c, 0)
            if c > 0:
                out_proj(c - 1, half=0)
            attention(c, 1)
            if c > 0:
                out_proj(c - 1, half=1)
        out_proj(NQB - 1)


_NC_CACHE = {}


def build_bass(do_compile=True):
    if do_compile in _NC_CACHE:
        return _NC_CACHE[do_compile]
    nc = bacc.Bacc()
    xh = nc.declare_dram_parameter("xh", [B, D, T], fp8, isOutput=False)
    xl = nc.declare_dram_parameter("xl", [B, D, T], fp8, isOutput=False)
    wqh = nc.declare_dram_parameter("wqh", [D, 3 * NH * HD], fp8, isOutput=False)
    wql = nc.declare_dram_parameter("wql", [D, 3 * NH * HD], fp8, isOutput=False)
    woh = nc.declare_dram_parameter("woh", [NH * HD, D], fp8, isOutput=False)
    wol = nc.declare_dram_parameter("wol", [NH * HD, D], fp8, isOutput=False)
    mask = nc.declare_dram_parameter("mask", [128, 128], f32, isOutput=False)
    y = nc.declare_dram_parameter("y", [B * T, D], f16, isOutput=True)
    with tile.TileContext(nc) as tc:
        with ExitStack() as ctx:
            with nc.allow_low_precision(
                    reason="fp8 hi/lo DoubleRow projections and bf16 PV; "
                           "validated against the f32 reference"):
                _body(ctx, tc, xh, xl, wqh, wql, woh, wol, mask, y[:, :])
    if do_compile:
        nc.compile()
    _NC_CACHE[do_compile] = nc
    return nc


def _hl(a):
    from ml_dtypes import float8_e4m3 as e4m3
    ah = np.clip(a, -240, 240).astype(e4m3)
    al = np.clip(a - ah.astype(np.float32), -240, 240).astype(e4m3)
    return ah, al


def shard_inputs(x, W_qkv, W_out):
    x = np.asarray(x, dtype=np.float32)
    W_qkv = np.asarray(W_qkv, dtype=np.float32)
    W_out = np.asarray(W_out, dtype=np.float32)

    xT = np.ascontiguousarray(x.transpose(0, 2, 1))          # [B, D, T]
    xh, xl = _hl(xT)
    i = np.arange(128)
    mask = np.where(i[:, None] <= i[None, :], 0.0, NEG).astype(np.float32)

    in_maps = []
    for c in range(N_CORES):
        r0 = c * NH * HD
        r1 = r0 + NH * HD
        wq = W_qkv[r0:r1].T                                   # [D, 256]
        wk = W_qkv[D + r0:D + r1].T
        wv = W_qkv[2 * D + r0:2 * D + r1].T
        wqkvT = np.concatenate([wq, wk, wv], axis=1) * SW
        wqh, wql = _hl(wqkvT)
        woT = W_out[:, r0:r1].T * SW                          # [256, D]
        woh, wol = _hl(woT)
        in_maps.append({"xh": xh, "xl": xl, "wqh": wqh, "wql": wql,
                        "woh": woh, "wol": wol, "mask": mask})
    return in_maps


def run(x, W_qkv, W_out, trace=False):
    nc = build_bass()
    in_maps = shard_inputs(x, W_qkv, W_out)
    res = run_bass_kernel_spmd(nc, in_maps, list(range(N_CORES)), trace=trace)
    parts = np.stack([np.asarray(r["y"], dtype=np.float32)
                      for r in res.results])                  # [8, B*T, D]
    y = parts.sum(axis=0)
    return y.reshape(B, T, D), res


def kernel(x, W_qkv, W_out):
    y, _ = run(x, W_qkv, W_out, trace=False)
    return y
